# revision 19
# baseline (speedup 1.0000x reference)
"""Trainium2 Bass kernel for 4-layer cross-stencil CNN.

Per-core: one image [6,256,256] (batch dim sharded across 8 cores).
conv(cross-5-stencil) = 5 channel-matmuls with spatially shifted rhs APs,
accumulated in PSUM. Channels on partitions, spatial (rows x cols) on the
free dim. fp32r matmuls (full PE rate at N>=256).

Strips of R output rows with overlap-compute for the halos; all four
layers fused in SBUF (no DRAM intermediates).

L1 packs the 5 taps into K=30 via a 5-group pre-shifted input buffer
(one matmul per chunk). L4 computes the 4 shifted taps as one M=128
matmul whose output slabs sit at partitions 0/32/64/96 (legal engine
bases), the center tap as an M=6 matmul, and the shifted tap-sum runs on
DVE in bf16; the tap-sum for strip k-1 is emitted interleaved into strip
k's L2 phase so the DVE FIFO never blocks the next strip's L1 copies.
"""

import sys

sys.path.insert(0, "/opt/trn_rl_repo")

import ml_dtypes
import numpy as np

import concourse.bacc as bacc
import concourse.mybir as mybir
from concourse.tile import TileContext
from concourse import bass_utils

IN_C, HID_C, OUT_C = 6, 128, 6
B, H, W = 8, 256, 256
WP = W + 2  # padded width
R = 24  # output rows per strip
N_CORES = 8

f32 = mybir.dt.float32
f32r = mybir.dt.float32r
bf16 = mybir.dt.bfloat16
Add = mybir.AluOpType.add
Max = mybir.AluOpType.max
Relu = mybir.ActivationFunctionType.Relu
Ident = mybir.ActivationFunctionType.Identity

# tap order matches reference: 0=center, 1=up(x[h-1]), 2=down(x[h+1]),
# 3=left(x[w-1]), 4=right(x[w+1])


def _build(repeat=1, stages=6, interleave=True):
    nc = bacc.Bacc("TRN2", target_bir_lowering=False)

    x_d = nc.dram_tensor("x", [IN_C, H, W], f32, kind="ExternalInput")
    w1_d = nc.dram_tensor("w1p", [5 * IN_C, HID_C], f32, kind="ExternalInput")
    w2_d = nc.dram_tensor("w2p", [HID_C, 5, HID_C], f32, kind="ExternalInput")
    w3_d = nc.dram_tensor("w3p", [HID_C, 5, HID_C], f32, kind="ExternalInput")
    # w4a: all 5 taps as M=128 slabs: up@0-5, center@6-11, down@32-37,
    # left@64-69, right@96-101; zero elsewhere
    w4a_d = nc.dram_tensor("w4a", [HID_C, HID_C], f32, kind="ExternalInput")
    # s6: bf16 selector summing the 5 (pre-shifted) slabs of t5s
    s6_d = nc.dram_tensor("s6", [HID_C, OUT_C], bf16, kind="ExternalInput")
    b1_d = nc.dram_tensor("b1", [HID_C], f32, kind="ExternalInput")
    b2_d = nc.dram_tensor("b2", [HID_C], f32, kind="ExternalInput")
    b3_d = nc.dram_tensor("b3", [HID_C], f32, kind="ExternalInput")
    b4_d = nc.dram_tensor("b4", [OUT_C], f32, kind="ExternalInput")
    y_d = nc.dram_tensor("y", [OUT_C, H, W], f32, kind="ExternalOutput")

    with TileContext(nc) as tc:
        with (
            tc.tile_pool(name="const", bufs=1) as cpool,
            tc.tile_pool(name="bufs", bufs=1) as bpool,
            tc.tile_pool(name="io", bufs=2) as iopool,
            tc.tile_pool(name="psmain", bufs=7, space="PSUM") as pmain,
        ):
            # --- weights / biases (resident) ---
            w1_sb = cpool.tile([5 * IN_C, HID_C], f32r)
            nc.sync.dma_start(out=w1_sb, in_=w1_d[:, :].bitcast(f32r))
            w2_sb = cpool.tile([HID_C, 5, HID_C], f32r)
            nc.sync.dma_start(out=w2_sb, in_=w2_d[:, :, :].bitcast(f32r))
            w3_sb = cpool.tile([HID_C, 5, HID_C], f32r)
            nc.sync.dma_start(out=w3_sb, in_=w3_d[:, :, :].bitcast(f32r))
            w4a_sb = cpool.tile([HID_C, HID_C], f32r)
            nc.sync.dma_start(out=w4a_sb, in_=w4a_d[:, :].bitcast(f32r))
            s6_sb = cpool.tile([HID_C, OUT_C], bf16)
            nc.sync.dma_start(out=s6_sb, in_=s6_d[:, :])
            b1_sb = cpool.tile([HID_C, 1], f32)
            nc.sync.dma_start(out=b1_sb, in_=b1_d[:, None])
            b2_sb = cpool.tile([HID_C, 1], f32)
            nc.sync.dma_start(out=b2_sb, in_=b2_d[:, None])
            b3_sb = cpool.tile([HID_C, 1], f32)
            nc.sync.dma_start(out=b3_sb, in_=b3_d[:, None])
            b4_sb = cpool.tile([OUT_C, 1], f32)
            nc.sync.dma_start(out=b4_sb, in_=b4_d[:, None])

            # --- persistent strip buffers (bufs=1; pads zeroed once) ---
            # x30: 5 tap-groups x 6ch, pre-shifted by DMA placement.
            # group g partitions [6g,6g+6); center x(h,w) -> (slot h-a+5, col w+1)
            x30 = bpool.tile([5 * IN_C, R + 10, WP], f32r)
            h1 = bpool.tile([HID_C, R + 6, WP], f32r)  # L1 rows [a-3,b+3)
            h2 = bpool.tile([HID_C, R + 4, WP], f32r)  # L2 rows [a-2,b+2)
            h3 = bpool.tile([HID_C, R + 2, WP], f32r)  # L3 rows [a-1,b+1)
            # t5: tap partials (slabs up@0,cen@6,dn@32,lf@64,rt@96), bf16
            t5 = bpool.tile([HID_C, R + 2, WP], bf16)
            # t5s: DMA-gathered pre-shifted taps; slot d = output row a+d
            t5s = bpool.tile([HID_C, R, WP], bf16)

            # zero only cells that are read but never written (all base-0 APs)
            nc.vector.memset(x30[:, 0:6, :].bitcast(f32), 0.0)
            nc.vector.memset(x30[:, :, 1:2].bitcast(f32), 0.0)
            nc.vector.memset(x30[:, :, 256:257].bitcast(f32), 0.0)
            for _t, _topz in ((h1, 3), (h2, 2), (h3, 1)):
                nc.vector.memset(_t[:, :, 0:1].bitcast(f32), 0.0)
                nc.vector.memset(_t[:, :, 257:258].bitcast(f32), 0.0)
                nc.vector.memset(_t[:, 0:_topz, :].bitcast(f32), 0.0)
            nc.vector.memset(t5[:, :, 0:1], 0.0)
            nc.vector.memset(t5[:, :, 257:258], 0.0)
            nc.vector.memset(t5[:, 0:1, :], 0.0)
            # t5s garbage partitions are never gathered; selector rows are
            # zero there but 0*garbage must not be NaN -> zero once
            nc.vector.memset(t5s[:, :, :], 0.0)

            def conv_chunk(ps, w_sb, src, s, n):
                """5 accumulating matmuls; center is src[:, s:s+n, 1:1+W]."""
                nc.tensor.matmul(
                    ps, w_sb[:, 0, :], src[:, s : s + n, 1 : 1 + W],
                    start=True, stop=False,
                )
                nc.tensor.matmul(
                    ps, w_sb[:, 1, :], src[:, s - 1 : s - 1 + n, 1 : 1 + W],
                    start=False, stop=False,
                )
                nc.tensor.matmul(
                    ps, w_sb[:, 2, :], src[:, s + 1 : s + 1 + n, 1 : 1 + W],
                    start=False, stop=False,
                )
                nc.tensor.matmul(
                    ps, w_sb[:, 3, :], src[:, s : s + n, 0:W],
                    start=False, stop=False,
                )
                nc.tensor.matmul(
                    ps, w_sb[:, 4, :], src[:, s : s + n, 2 : 2 + W],
                    start=False, stop=True,
                )

            def l4b_chunks(a0, b0):
                """Deferred emitters: one K=128 selector matmul sums the 5
                pre-shifted slabs of t5s; bias on DVE; DMA out."""
                out = []
                rr = a0
                while rr < b0:
                    n = min(2, b0 - rr)

                    def emit(rr=rr, n=n, a0=a0):
                        d = rr - a0
                        ps = pmain.tile([OUT_C, n, W], f32, tag="ps")
                        nc.tensor.matmul(
                            ps, s6_sb[:, :], t5s[:, d : d + n, 1 : 1 + W],
                            start=True, stop=True,
                        )
                        yt = iopool.tile([OUT_C, n, W], f32, tag="yt")
                        nc.vector.tensor_scalar_add(yt, ps, b4_sb)
                        nc.scalar.dma_start(out=y_d[:, rr : rr + n, :], in_=yt)

                    out.append(emit)
                    rr += n
                return out

            pending = []  # tap-sum emitters from the previous strip
            for rep in range(repeat):
             for a in range(0, H, R):
                b = min(a + R, H)
                last = b == H
                lo_x, hi_x = max(0, a - 4), min(H, b + 4)

                if last:
                    # re-zero stale tail slots (bufs=1 reuse) before writes
                    nc.vector.memset(
                        x30[:, hi_x - a + 4 : R + 10, :].bitcast(f32), 0.0)
                    nc.vector.memset(
                        h1[:, 256 - (a - 3) : R + 6, :].bitcast(f32), 0.0)
                    nc.vector.memset(
                        h2[:, 256 - (a - 2) : R + 4, :].bitcast(f32), 0.0)
                    nc.vector.memset(
                        h3[:, 256 - (a - 1) : R + 2, :].bitcast(f32), 0.0)

                # --- load x strip, 5 shifted placements ---
                src = x_d[:, lo_x:hi_x, :].bitcast(f32r)
                o = lo_x - a
                nc.sync.dma_start(out=x30[0:6, o + 5 : hi_x - a + 5, 1 : 1 + W], in_=src)
                nc.sync.dma_start(out=x30[6:12, o + 6 : hi_x - a + 6, 1 : 1 + W], in_=src)
                nc.sync.dma_start(out=x30[12:18, o + 4 : hi_x - a + 4, 1 : 1 + W], in_=src)
                nc.sync.dma_start(out=x30[18:24, o + 5 : hi_x - a + 5, 2 : 2 + W], in_=src)
                nc.sync.dma_start(out=x30[24:30, o + 5 : hi_x - a + 5, 0:W], in_=src)

                # --- L1: rows [a-3, b+3) -> h1; copies alternate DVE/ACT ---
                rr = max(0, a - 3)
                hi = min(H, b + 3)
                ci = 0
                while rr < hi:
                    n = min(2, hi - rr)
                    s = rr - a + 5
                    ps = pmain.tile([HID_C, n, W], f32, tag="ps")
                    nc.tensor.matmul(
                        ps, w1_sb[:, :], x30[:, s : s + n, 1 : 1 + W],
                        start=True, stop=True,
                    )
                    d = rr - (a - 3)
                    if ci % 2 == 0:
                        nc.vector.tensor_scalar(
                            h1[:, d : d + n, 1 : 1 + W], ps, b1_sb, 0.0, Add, Max
                        )
                    else:
                        nc.scalar.activation(
                            h1[:, d : d + n, 1 : 1 + W], ps, Relu, bias=b1_sb
                        )
                    ci += 1
                    rr += n

                # --- L2: rows [a-2, b+2), reads h1; interleave prev tap-sum ---
                rr = max(0, a - 2) if stages >= 2 else hi
                hi = min(H, b + 2)
                while rr < hi:
                    n = min(2, hi - rr)
                    s = rr - a + 3  # h1 slot of center
                    ps = pmain.tile([HID_C, n, W], f32, tag="ps")
                    conv_chunk(ps, w2_sb, h1, s, n)
                    d = rr - (a - 2)
                    nc.scalar.activation(
                        h2[:, d : d + n, 1 : 1 + W], ps, Relu, bias=b2_sb
                    )
                    if pending and interleave:
                        pending.pop(0)()
                    rr += n
                while pending and interleave:
                    pending.pop(0)()

                # --- L3: rows [a-1, b+1), reads h2 ---
                rr = max(0, a - 1) if stages >= 3 else H
                hi = min(H, b + 1)
                while rr < hi:
                    n = min(2, hi - rr)
                    s = rr - a + 2  # h2 slot of center
                    ps = pmain.tile([HID_C, n, W], f32, tag="ps")
                    conv_chunk(ps, w3_sb, h2, s, n)
                    d = rr - (a - 1)
                    nc.scalar.activation(
                        h3[:, d : d + n, 1 : 1 + W], ps, Relu, bias=b3_sb
                    )
                    rr += n

                # --- L4a: shifted-tap partials (M=128, 4 slabs), bf16 ---
                if stages < 4:
                    continue
                # tail slots beyond the written range must be zero; emitted
                # here (after the previous strip's tap-sum drained) to avoid
                # clobbering t5 while deferred reads are pending
                lo4, hi4 = max(0, a - 1), min(H, b + 1)
                if hi4 - (a - 1) < R + 2:
                    nc.vector.memset(t5[:, hi4 - (a - 1) : R + 2, :], 0.0)
                rr = lo4
                ci = 0
                while rr < hi4:
                    n = min(2, hi4 - rr)
                    s = rr - a + 1  # h3 slot of center
                    ps = pmain.tile([HID_C, n, W], f32, tag="ps")
                    nc.tensor.matmul(
                        ps, w4a_sb[:, :], h3[:, s : s + n, 1 : 1 + W],
                        start=True, stop=True,
                    )
                    d = rr - (a - 1)  # t5 slot
                    if ci % 2 == 0:
                        nc.vector.tensor_copy(t5[:, d : d + n, 1 : 1 + W], ps)
                    else:
                        nc.scalar.activation(
                            t5[:, d : d + n, 1 : 1 + W], ps, Ident
                        )
                    ci += 1
                    rr += n

                if stages < 5:
                    continue
                # --- gather: build pre-shifted t5s via SBUF->SBUF DMA ---
                # Full-padded-row flat copies: one contiguous run per
                # partition. t5 slot st = row-(a-1); t5s slot d = row-a.
                nr = b - a
                L = nr * WP
                t5f = t5.rearrange("p r c -> p (r c)")
                t5sf = t5s.rearrange("p r c -> p (r c)")
                nc.scalar.dma_start(out=t5sf[0:6, 0:L], in_=t5f[0:6, 0:L])
                nc.scalar.dma_start(out=t5sf[6:12, 0:L], in_=t5f[6:12, WP : WP + L])
                nc.scalar.dma_start(
                    out=t5sf[32:38, 0:L], in_=t5f[32:38, 2 * WP : 2 * WP + L])
                nc.scalar.dma_start(
                    out=t5sf[64:70, 1:L], in_=t5f[64:70, WP : WP + L - 1])
                nc.scalar.dma_start(
                    out=t5sf[96:102, 0 : L - 1], in_=t5f[96:102, WP + 1 : WP + L])

                if stages < 6:
                    continue
                if not interleave:
                    while pending:
                        pending.pop(0)()
                pending = l4b_chunks(a, b)

            # flush the final strip's tap-sum
            while pending:
                pending.pop(0)()

    nc.finalize()
    return nc


_NC_CACHE = {}


def _pack_inputs(x, w1, b1, w2, b2, w3, b3, w4, b4):
    x = np.ascontiguousarray(np.asarray(x, dtype=np.float32))
    w1 = np.asarray(w1, dtype=np.float32)
    w2 = np.asarray(w2, dtype=np.float32)
    w3 = np.asarray(w3, dtype=np.float32)
    w4 = np.asarray(w4, dtype=np.float32)
    # w4a slabs: up@0-5, center@6-11, down@32-37, left@64-69, right@96-101
    w4a = np.zeros((HID_C, HID_C), np.float32)
    slabs = ((0, 1), (6, 2 - 2), (32, 2), (64, 3), (96, 4))
    w4a[:, 0:OUT_C] = w4[:, :, 1].T          # up
    w4a[:, 6 : 6 + OUT_C] = w4[:, :, 0].T    # center
    w4a[:, 32 : 32 + OUT_C] = w4[:, :, 2].T  # down
    w4a[:, 64 : 64 + OUT_C] = w4[:, :, 3].T  # left
    w4a[:, 96 : 96 + OUT_C] = w4[:, :, 4].T  # right
    s6 = np.zeros((HID_C, OUT_C), np.float32)
    for base in (0, 6, 32, 64, 96):
        s6[base + np.arange(OUT_C), np.arange(OUT_C)] = 1.0
    s6 = s6.astype(ml_dtypes.bfloat16)
    common = {
        # w1p[t*6+ic, oc] = w1[oc, ic, t]
        "w1p": np.ascontiguousarray(w1.transpose(2, 1, 0).reshape(5 * IN_C, HID_C)),
        # w2p[ic, t, oc] = w2[oc, ic, t]
        "w2p": np.ascontiguousarray(w2.transpose(1, 2, 0)),
        "w3p": np.ascontiguousarray(w3.transpose(1, 2, 0)),
        "w4a": w4a,
        "s6": s6,
        "b1": np.asarray(b1, np.float32),
        "b2": np.asarray(b2, np.float32),
        "b3": np.asarray(b3, np.float32),
        "b4": np.asarray(b4, np.float32),
    }
    return x, common


def kernel(x, w1, b1, w2, b2, w3, b3, w4, b4):
    x, common = _pack_inputs(x, w1, b1, w2, b2, w3, b3, w4, b4)
    if "nc" not in _NC_CACHE:
        _NC_CACHE["nc"] = _build()
    nc = _NC_CACHE["nc"]
    in_maps = [dict(common, x=x[i]) for i in range(N_CORES)]
    res = bass_utils.run_bass_kernel_spmd(nc, in_maps, core_ids=list(range(N_CORES)))
    out = np.stack([res.results[i]["y"] for i in range(N_CORES)], axis=0)
    return out


# revision 20
# speedup vs baseline: 2.9366x; 2.9366x over previous
"""Trainium2 Bass kernel for 4-layer cross-stencil CNN.

Per-core: one image [6,256,256] (batch dim sharded across 8 cores).
conv(cross-5-stencil) = 5 channel-matmuls with spatially shifted rhs APs,
accumulated in PSUM. Channels on partitions, spatial (rows x cols) on the
free dim. fp32r matmuls (full PE rate at N>=256).

Strips of R output rows with overlap-compute for the halos; all four
layers fused in SBUF (no DRAM intermediates).

L1 packs the 5 taps into K=30 via a 5-group pre-shifted input buffer
(one matmul per chunk). L4 computes the 4 shifted taps as one M=128
matmul whose output slabs sit at partitions 0/32/64/96 (legal engine
bases), the center tap as an M=6 matmul, and the shifted tap-sum runs on
DVE in bf16; the tap-sum for strip k-1 is emitted interleaved into strip
k's L2 phase so the DVE FIFO never blocks the next strip's L1 copies.
"""

import sys

sys.path.insert(0, "/opt/trn_rl_repo")

import ml_dtypes
import numpy as np

import concourse.bacc as bacc
import concourse.mybir as mybir
from concourse.tile import TileContext
from concourse import bass_utils

IN_C, HID_C, OUT_C = 6, 128, 6
B, H, W = 8, 256, 256
WP = W + 2  # padded width
R = 24  # output rows per strip
N_CORES = 8

f32 = mybir.dt.float32
f32r = mybir.dt.float32r
bf16 = mybir.dt.bfloat16
Add = mybir.AluOpType.add
Max = mybir.AluOpType.max
Relu = mybir.ActivationFunctionType.Relu
Ident = mybir.ActivationFunctionType.Identity

# tap order matches reference: 0=center, 1=up(x[h-1]), 2=down(x[h+1]),
# 3=left(x[w-1]), 4=right(x[w+1])


def _build(repeat=1, stages=6, interleave=True):
    nc = bacc.Bacc("TRN2", target_bir_lowering=False)

    x_d = nc.dram_tensor("x", [IN_C, H, W], f32, kind="ExternalInput")
    w1_d = nc.dram_tensor("w1p", [5 * IN_C, HID_C], f32, kind="ExternalInput")
    w2_d = nc.dram_tensor("w2p", [HID_C, 5, HID_C], f32, kind="ExternalInput")
    w3_d = nc.dram_tensor("w3p", [HID_C, 5, HID_C], f32, kind="ExternalInput")
    # w4a: all 5 taps as M=128 slabs: up@0-5, center@6-11, down@32-37,
    # left@64-69, right@96-101; zero elsewhere
    w4a_d = nc.dram_tensor("w4a", [HID_C, HID_C], f32, kind="ExternalInput")
    # s6: bf16 selector summing the 5 (pre-shifted) slabs of t5s
    s6_d = nc.dram_tensor("s6", [HID_C, OUT_C], bf16, kind="ExternalInput")
    b1_d = nc.dram_tensor("b1", [HID_C], f32, kind="ExternalInput")
    b2_d = nc.dram_tensor("b2", [HID_C], f32, kind="ExternalInput")
    b3_d = nc.dram_tensor("b3", [HID_C], f32, kind="ExternalInput")
    b4_d = nc.dram_tensor("b4", [OUT_C], f32, kind="ExternalInput")
    y_d = nc.dram_tensor("y", [OUT_C, H, W], f32, kind="ExternalOutput")

    with TileContext(nc) as tc:
        with (
            tc.tile_pool(name="const", bufs=1) as cpool,
            tc.tile_pool(name="bufs", bufs=1) as bpool,
            tc.tile_pool(name="io", bufs=3) as iopool,
            tc.tile_pool(name="psmain", bufs=7, space="PSUM") as pmain,
        ):
            # --- weights / biases (resident) ---
            w1_sb = cpool.tile([5 * IN_C, HID_C], f32r)
            nc.sync.dma_start(out=w1_sb, in_=w1_d[:, :].bitcast(f32r))
            w2_sb = cpool.tile([HID_C, 5, HID_C], f32r)
            nc.sync.dma_start(out=w2_sb, in_=w2_d[:, :, :].bitcast(f32r))
            w3_sb = cpool.tile([HID_C, 5, HID_C], f32r)
            nc.sync.dma_start(out=w3_sb, in_=w3_d[:, :, :].bitcast(f32r))
            w4a_sb = cpool.tile([HID_C, HID_C], f32r)
            nc.sync.dma_start(out=w4a_sb, in_=w4a_d[:, :].bitcast(f32r))
            s6_sb = cpool.tile([HID_C, OUT_C], bf16)
            nc.sync.dma_start(out=s6_sb, in_=s6_d[:, :])
            b1_sb = cpool.tile([HID_C, 1], f32)
            nc.sync.dma_start(out=b1_sb, in_=b1_d[:, None])
            b2_sb = cpool.tile([HID_C, 1], f32)
            nc.sync.dma_start(out=b2_sb, in_=b2_d[:, None])
            b3_sb = cpool.tile([HID_C, 1], f32)
            nc.sync.dma_start(out=b3_sb, in_=b3_d[:, None])
            b4_sb = cpool.tile([OUT_C, 1], f32)
            nc.sync.dma_start(out=b4_sb, in_=b4_d[:, None])

            # --- persistent strip buffers (bufs=1; pads zeroed once) ---
            # x30: 5 tap-groups x 6ch, pre-shifted by DMA placement.
            # group g partitions [6g,6g+6); center x(h,w) -> (slot h-a+5, col w+1)
            x30 = bpool.tile([5 * IN_C, R + 10, WP], f32r)
            h1 = bpool.tile([HID_C, R + 6, WP], f32r)  # L1 rows [a-3,b+3)
            h2 = bpool.tile([HID_C, R + 4, WP], f32r)  # L2 rows [a-2,b+2)
            h3 = bpool.tile([HID_C, R + 2, WP], f32r)  # L3 rows [a-1,b+1)
            # t5: tap partials (slabs up@0,cen@6,dn@32,lf@64,rt@96), bf16
            t5 = bpool.tile([HID_C, R + 2, WP], bf16)
            # t5s: DMA-gathered pre-shifted taps; slot d = output row a+d
            t5s = bpool.tile([HID_C, R, WP], bf16)

            # zero only cells that are read but never written (all base-0 APs)
            nc.vector.memset(x30[:, 0:6, :].bitcast(f32), 0.0)
            nc.vector.memset(x30[:, :, 1:2].bitcast(f32), 0.0)
            nc.vector.memset(x30[:, :, 256:257].bitcast(f32), 0.0)
            for _t, _topz in ((h1, 3), (h2, 2), (h3, 1)):
                nc.vector.memset(_t[:, :, 0:1].bitcast(f32), 0.0)
                nc.vector.memset(_t[:, :, 257:258].bitcast(f32), 0.0)
                nc.vector.memset(_t[:, 0:_topz, :].bitcast(f32), 0.0)
            nc.vector.memset(t5[:, :, 0:1], 0.0)
            nc.vector.memset(t5[:, :, 257:258], 0.0)
            nc.vector.memset(t5[:, 0:1, :], 0.0)
            # t5s garbage partitions are never gathered; selector rows are
            # zero there but 0*garbage must not be NaN -> zero once
            nc.vector.memset(t5s[:, :, :], 0.0)

            def conv_chunk(ps, w_sb, src, s, n):
                """5 accumulating matmuls; center is src[:, s:s+n, 1:1+W]."""
                nc.tensor.matmul(
                    ps, w_sb[:, 0, :], src[:, s : s + n, 1 : 1 + W],
                    start=True, stop=False,
                )
                nc.tensor.matmul(
                    ps, w_sb[:, 1, :], src[:, s - 1 : s - 1 + n, 1 : 1 + W],
                    start=False, stop=False,
                )
                nc.tensor.matmul(
                    ps, w_sb[:, 2, :], src[:, s + 1 : s + 1 + n, 1 : 1 + W],
                    start=False, stop=False,
                )
                nc.tensor.matmul(
                    ps, w_sb[:, 3, :], src[:, s : s + n, 0:W],
                    start=False, stop=False,
                )
                nc.tensor.matmul(
                    ps, w_sb[:, 4, :], src[:, s : s + n, 2 : 2 + W],
                    start=False, stop=True,
                )

            def l4b_chunks(a0, b0):
                """Deferred emitters: one K=128 selector matmul sums the 5
                pre-shifted slabs of t5s; bias on DVE; DMA out."""
                out = []
                rr = a0
                while rr < b0:
                    n = min(2, b0 - rr)

                    def emit(rr=rr, n=n, a0=a0):
                        d = rr - a0
                        ps = pmain.tile([OUT_C, n, W], f32, tag="ps")
                        nc.tensor.matmul(
                            ps, s6_sb[:, :], t5s[:, d : d + n, 1 : 1 + W],
                            start=True, stop=True,
                        )
                        yt = iopool.tile([OUT_C, n, W], f32, tag="yt")
                        nc.vector.tensor_scalar_add(yt, ps, b4_sb)
                        nc.scalar.dma_start(out=y_d[:, rr : rr + n, :], in_=yt)

                    out.append(emit)
                    rr += n
                return out

            pending = []  # tap-sum emitters from the previous strip
            for rep in range(repeat):
             for a in range(0, H, R):
                b = min(a + R, H)
                last = b == H
                lo_x, hi_x = max(0, a - 4), min(H, b + 4)

                if last:
                    # re-zero stale tail slots (bufs=1 reuse) before writes
                    nc.vector.memset(
                        x30[:, hi_x - a + 4 : R + 10, :].bitcast(f32), 0.0)
                    nc.vector.memset(
                        h1[:, 256 - (a - 3) : R + 6, :].bitcast(f32), 0.0)
                    nc.vector.memset(
                        h2[:, 256 - (a - 2) : R + 4, :].bitcast(f32), 0.0)
                    nc.vector.memset(
                        h3[:, 256 - (a - 1) : R + 2, :].bitcast(f32), 0.0)

                # --- load x strip, 5 shifted placements ---
                src = x_d[:, lo_x:hi_x, :].bitcast(f32r)
                o = lo_x - a
                nc.sync.dma_start(out=x30[0:6, o + 5 : hi_x - a + 5, 1 : 1 + W], in_=src)
                nc.sync.dma_start(out=x30[6:12, o + 6 : hi_x - a + 6, 1 : 1 + W], in_=src)
                nc.sync.dma_start(out=x30[12:18, o + 4 : hi_x - a + 4, 1 : 1 + W], in_=src)
                nc.sync.dma_start(out=x30[18:24, o + 5 : hi_x - a + 5, 2 : 2 + W], in_=src)
                nc.sync.dma_start(out=x30[24:30, o + 5 : hi_x - a + 5, 0:W], in_=src)

                # --- L1: rows [a-3, b+3) -> h1; copies alternate DVE/ACT ---
                rr = max(0, a - 3)
                hi = min(H, b + 3)
                ci = 0
                while rr < hi:
                    n = min(2, hi - rr)
                    s = rr - a + 5
                    ps = pmain.tile([HID_C, n, W], f32, tag="ps")
                    nc.tensor.matmul(
                        ps, w1_sb[:, :], x30[:, s : s + n, 1 : 1 + W],
                        start=True, stop=True,
                    )
                    d = rr - (a - 3)
                    if ci % 2 == 0:
                        nc.vector.tensor_scalar(
                            h1[:, d : d + n, 1 : 1 + W], ps, b1_sb, 0.0, Add, Max
                        )
                    else:
                        nc.scalar.activation(
                            h1[:, d : d + n, 1 : 1 + W], ps, Relu, bias=b1_sb
                        )
                    ci += 1
                    rr += n

                # --- L2: rows [a-2, b+2), reads h1; interleave prev tap-sum ---
                rr = max(0, a - 2) if stages >= 2 else hi
                hi = min(H, b + 2)
                while rr < hi:
                    n = min(2, hi - rr)
                    s = rr - a + 3  # h1 slot of center
                    ps = pmain.tile([HID_C, n, W], f32, tag="ps")
                    conv_chunk(ps, w2_sb, h1, s, n)
                    d = rr - (a - 2)
                    nc.scalar.activation(
                        h2[:, d : d + n, 1 : 1 + W], ps, Relu, bias=b2_sb
                    )
                    if pending and interleave:
                        pending.pop(0)()
                    rr += n
                while pending and interleave:
                    pending.pop(0)()

                # --- L3: rows [a-1, b+1), reads h2 ---
                rr = max(0, a - 1) if stages >= 3 else H
                hi = min(H, b + 1)
                while rr < hi:
                    n = min(2, hi - rr)
                    s = rr - a + 2  # h2 slot of center
                    ps = pmain.tile([HID_C, n, W], f32, tag="ps")
                    conv_chunk(ps, w3_sb, h2, s, n)
                    d = rr - (a - 1)
                    nc.scalar.activation(
                        h3[:, d : d + n, 1 : 1 + W], ps, Relu, bias=b3_sb
                    )
                    rr += n

                # --- L4a: shifted-tap partials (M=128, 4 slabs), bf16 ---
                if stages < 4:
                    continue
                # tail slots beyond the written range must be zero; emitted
                # here (after the previous strip's tap-sum drained) to avoid
                # clobbering t5 while deferred reads are pending
                lo4, hi4 = max(0, a - 1), min(H, b + 1)
                if hi4 - (a - 1) < R + 2:
                    nc.vector.memset(t5[:, hi4 - (a - 1) : R + 2, :], 0.0)
                rr = lo4
                ci = 0
                while rr < hi4:
                    n = min(2, hi4 - rr)
                    s = rr - a + 1  # h3 slot of center
                    ps = pmain.tile([HID_C, n, W], f32, tag="ps")
                    nc.tensor.matmul(
                        ps, w4a_sb[:, :], h3[:, s : s + n, 1 : 1 + W],
                        start=True, stop=True,
                    )
                    d = rr - (a - 1)  # t5 slot
                    if ci % 2 == 0:
                        nc.vector.tensor_copy(t5[:, d : d + n, 1 : 1 + W], ps)
                    else:
                        nc.scalar.activation(
                            t5[:, d : d + n, 1 : 1 + W], ps, Ident
                        )
                    ci += 1
                    rr += n

                if stages < 5:
                    continue
                # --- gather: build pre-shifted t5s via SBUF->SBUF DMA ---
                # Full-padded-row flat copies: one contiguous run per
                # partition. t5 slot st = row-(a-1); t5s slot d = row-a.
                nr = b - a
                L = nr * WP
                t5f = t5.rearrange("p r c -> p (r c)")
                t5sf = t5s.rearrange("p r c -> p (r c)")
                nc.scalar.dma_start(out=t5sf[0:6, 0:L], in_=t5f[0:6, 0:L])
                nc.scalar.dma_start(out=t5sf[6:12, 0:L], in_=t5f[6:12, WP : WP + L])
                nc.scalar.dma_start(
                    out=t5sf[32:38, 0:L], in_=t5f[32:38, 2 * WP : 2 * WP + L])
                nc.scalar.dma_start(
                    out=t5sf[64:70, 1:L], in_=t5f[64:70, WP : WP + L - 1])
                nc.scalar.dma_start(
                    out=t5sf[96:102, 0 : L - 1], in_=t5f[96:102, WP + 1 : WP + L])

                if stages < 6:
                    continue
                if not interleave:
                    while pending:
                        pending.pop(0)()
                pending = l4b_chunks(a, b)

            # flush the final strip's tap-sum
            while pending:
                pending.pop(0)()

    nc.finalize()
    return nc


_NC_CACHE = {}


def _pack_inputs(x, w1, b1, w2, b2, w3, b3, w4, b4):
    x = np.ascontiguousarray(np.asarray(x, dtype=np.float32))
    w1 = np.asarray(w1, dtype=np.float32)
    w2 = np.asarray(w2, dtype=np.float32)
    w3 = np.asarray(w3, dtype=np.float32)
    w4 = np.asarray(w4, dtype=np.float32)
    # w4a slabs: up@0-5, center@6-11, down@32-37, left@64-69, right@96-101
    w4a = np.zeros((HID_C, HID_C), np.float32)
    slabs = ((0, 1), (6, 2 - 2), (32, 2), (64, 3), (96, 4))
    w4a[:, 0:OUT_C] = w4[:, :, 1].T          # up
    w4a[:, 6 : 6 + OUT_C] = w4[:, :, 0].T    # center
    w4a[:, 32 : 32 + OUT_C] = w4[:, :, 2].T  # down
    w4a[:, 64 : 64 + OUT_C] = w4[:, :, 3].T  # left
    w4a[:, 96 : 96 + OUT_C] = w4[:, :, 4].T  # right
    s6 = np.zeros((HID_C, OUT_C), np.float32)
    for base in (0, 6, 32, 64, 96):
        s6[base + np.arange(OUT_C), np.arange(OUT_C)] = 1.0
    s6 = s6.astype(ml_dtypes.bfloat16)
    common = {
        # w1p[t*6+ic, oc] = w1[oc, ic, t]
        "w1p": np.ascontiguousarray(w1.transpose(2, 1, 0).reshape(5 * IN_C, HID_C)),
        # w2p[ic, t, oc] = w2[oc, ic, t]
        "w2p": np.ascontiguousarray(w2.transpose(1, 2, 0)),
        "w3p": np.ascontiguousarray(w3.transpose(1, 2, 0)),
        "w4a": w4a,
        "s6": s6,
        "b1": np.asarray(b1, np.float32),
        "b2": np.asarray(b2, np.float32),
        "b3": np.asarray(b3, np.float32),
        "b4": np.asarray(b4, np.float32),
    }
    return x, common


def kernel(x, w1, b1, w2, b2, w3, b3, w4, b4):
    x, common = _pack_inputs(x, w1, b1, w2, b2, w3, b3, w4, b4)
    if "nc" not in _NC_CACHE:
        _NC_CACHE["nc"] = _build()
    nc = _NC_CACHE["nc"]
    in_maps = [dict(common, x=x[i]) for i in range(N_CORES)]
    res = bass_utils.run_bass_kernel_spmd(nc, in_maps, core_ids=list(range(N_CORES)))
    out = np.stack([res.results[i]["y"] for i in range(N_CORES)], axis=0)
    return out


# revision 21
# speedup vs baseline: 2.9936x; 1.0194x over previous
"""Trainium2 Bass kernel for 4-layer cross-stencil CNN.

Per-core: one image [6,256,256] (batch dim sharded across 8 cores).
conv(cross-5-stencil) = 5 channel-matmuls with spatially shifted rhs APs,
accumulated in PSUM. Channels on partitions, spatial (rows x cols) on the
free dim. fp32r matmuls (full PE rate at N>=256).

Strips of R output rows with overlap-compute for the halos; all four
layers fused in SBUF (no DRAM intermediates).

L1 packs the 5 taps into K=30 via a 5-group pre-shifted input buffer
(one matmul per chunk). L4 computes the 4 shifted taps as one M=128
matmul whose output slabs sit at partitions 0/32/64/96 (legal engine
bases), the center tap as an M=6 matmul, and the shifted tap-sum runs on
DVE in bf16; the tap-sum for strip k-1 is emitted interleaved into strip
k's L2 phase so the DVE FIFO never blocks the next strip's L1 copies.
"""

import sys

sys.path.insert(0, "/opt/trn_rl_repo")

import ml_dtypes
import numpy as np

import concourse.bacc as bacc
import concourse.mybir as mybir
from concourse.tile import TileContext
from concourse import bass_utils

IN_C, HID_C, OUT_C = 6, 128, 6
B, H, W = 8, 256, 256
WP = W + 2  # padded width
R = 24  # output rows per strip
N_CORES = 8

f32 = mybir.dt.float32
f32r = mybir.dt.float32r
bf16 = mybir.dt.bfloat16
Add = mybir.AluOpType.add
Max = mybir.AluOpType.max
Relu = mybir.ActivationFunctionType.Relu
Ident = mybir.ActivationFunctionType.Identity

# tap order matches reference: 0=center, 1=up(x[h-1]), 2=down(x[h+1]),
# 3=left(x[w-1]), 4=right(x[w+1])


def _build(repeat=1, stages=6, interleave=True):
    nc = bacc.Bacc("TRN2", target_bir_lowering=False)

    x_d = nc.dram_tensor("x", [IN_C, H, W], f32, kind="ExternalInput")
    w1_d = nc.dram_tensor("w1p", [5 * IN_C, HID_C], f32, kind="ExternalInput")
    w2_d = nc.dram_tensor("w2p", [HID_C, 5, HID_C], f32, kind="ExternalInput")
    w3_d = nc.dram_tensor("w3p", [HID_C, 5, HID_C], f32, kind="ExternalInput")
    # w4a: all 5 taps as M=128 slabs: up@0-5, center@6-11, down@32-37,
    # left@64-69, right@96-101; zero elsewhere
    w4a_d = nc.dram_tensor("w4a", [HID_C, HID_C], f32, kind="ExternalInput")
    # s6: bf16 selector summing the 5 (pre-shifted) slabs of t5s
    s6_d = nc.dram_tensor("s6", [HID_C, OUT_C], bf16, kind="ExternalInput")
    b1_d = nc.dram_tensor("b1", [HID_C], f32, kind="ExternalInput")
    b2_d = nc.dram_tensor("b2", [HID_C], f32, kind="ExternalInput")
    b3_d = nc.dram_tensor("b3", [HID_C], f32, kind="ExternalInput")
    b4_d = nc.dram_tensor("b4", [OUT_C], f32, kind="ExternalInput")
    y_d = nc.dram_tensor("y", [OUT_C, H, W], f32, kind="ExternalOutput")

    with TileContext(nc) as tc:
        with (
            tc.tile_pool(name="const", bufs=1) as cpool,
            tc.tile_pool(name="bufs", bufs=1) as bpool,
            tc.tile_pool(name="io", bufs=3) as iopool,
            tc.tile_pool(name="psmain", bufs=7, space="PSUM") as pmain,
        ):
            # --- weights / biases (resident) ---
            w1_sb = cpool.tile([5 * IN_C, HID_C], f32r)
            nc.sync.dma_start(out=w1_sb, in_=w1_d[:, :].bitcast(f32r))
            w2_sb = cpool.tile([HID_C, 5, HID_C], f32r)
            nc.sync.dma_start(out=w2_sb, in_=w2_d[:, :, :].bitcast(f32r))
            w3_sb = cpool.tile([HID_C, 5, HID_C], f32r)
            nc.sync.dma_start(out=w3_sb, in_=w3_d[:, :, :].bitcast(f32r))
            w4a_sb = cpool.tile([HID_C, HID_C], f32r)
            nc.sync.dma_start(out=w4a_sb, in_=w4a_d[:, :].bitcast(f32r))
            s6_sb = cpool.tile([HID_C, OUT_C], bf16)
            nc.sync.dma_start(out=s6_sb, in_=s6_d[:, :])
            b1_sb = cpool.tile([HID_C, 1], f32)
            nc.sync.dma_start(out=b1_sb, in_=b1_d[:, None])
            b2_sb = cpool.tile([HID_C, 1], f32)
            nc.sync.dma_start(out=b2_sb, in_=b2_d[:, None])
            b3_sb = cpool.tile([HID_C, 1], f32)
            nc.sync.dma_start(out=b3_sb, in_=b3_d[:, None])
            b4_sb = cpool.tile([OUT_C, 1], f32)
            nc.sync.dma_start(out=b4_sb, in_=b4_d[:, None])

            # --- persistent strip buffers (bufs=1; pads zeroed once) ---
            # x30: 5 tap-groups x 6ch, pre-shifted by DMA placement.
            # group g partitions [6g,6g+6); center x(h,w) -> (slot h-a+5, col w+1)
            x30 = bpool.tile([5 * IN_C, R + 10, WP], f32r)
            h1 = bpool.tile([HID_C, R + 6, WP], f32r)  # L1 rows [a-3,b+3)
            h2 = bpool.tile([HID_C, R + 4, WP], f32r)  # L2 rows [a-2,b+2)
            h3 = bpool.tile([HID_C, R + 2, WP], f32r)  # L3 rows [a-1,b+1)
            # t5: tap partials (slabs up@0,cen@6,dn@32,lf@64,rt@96), bf16
            t5 = bpool.tile([HID_C, R + 2, WP], bf16)
            # t5s: DMA-gathered pre-shifted taps; slot d = output row a+d
            t5s = bpool.tile([HID_C, R, WP], bf16)

            # zero only cells that are read but never written (all base-0 APs)
            nc.vector.memset(x30[:, 0:6, :].bitcast(f32), 0.0)
            nc.vector.memset(x30[:, :, 1:2].bitcast(f32), 0.0)
            nc.vector.memset(x30[:, :, 256:257].bitcast(f32), 0.0)
            for _t, _topz in ((h1, 3), (h2, 2), (h3, 1)):
                nc.vector.memset(_t[:, :, 0:1].bitcast(f32), 0.0)
                nc.vector.memset(_t[:, :, 257:258].bitcast(f32), 0.0)
                nc.vector.memset(_t[:, 0:_topz, :].bitcast(f32), 0.0)
            nc.vector.memset(t5[:, :, 0:1], 0.0)
            nc.vector.memset(t5[:, :, 257:258], 0.0)
            nc.vector.memset(t5[:, 0:1, :], 0.0)
            # t5s garbage partitions are never gathered; selector rows are
            # zero there but 0*garbage must not be NaN -> zero once
            nc.vector.memset(t5s[:, :, :], 0.0)

            def conv_chunk(ps, w_sb, src, s, n):
                """5 accumulating matmuls; center is src[:, s:s+n, 1:1+W]."""
                nc.tensor.matmul(
                    ps, w_sb[:, 0, :], src[:, s : s + n, 1 : 1 + W],
                    start=True, stop=False,
                )
                nc.tensor.matmul(
                    ps, w_sb[:, 1, :], src[:, s - 1 : s - 1 + n, 1 : 1 + W],
                    start=False, stop=False,
                )
                nc.tensor.matmul(
                    ps, w_sb[:, 2, :], src[:, s + 1 : s + 1 + n, 1 : 1 + W],
                    start=False, stop=False,
                )
                nc.tensor.matmul(
                    ps, w_sb[:, 3, :], src[:, s : s + n, 0:W],
                    start=False, stop=False,
                )
                nc.tensor.matmul(
                    ps, w_sb[:, 4, :], src[:, s : s + n, 2 : 2 + W],
                    start=False, stop=True,
                )

            def l4b_chunks(a0, b0):
                """Deferred emitters: one K=128 selector matmul sums the 5
                pre-shifted slabs of t5s; bias on DVE; DMA out."""
                out = []
                rr = a0
                while rr < b0:
                    n = min(2, b0 - rr)

                    def emit(rr=rr, n=n, a0=a0):
                        d = rr - a0
                        ps = pmain.tile([OUT_C, n, W], f32, tag="ps")
                        nc.tensor.matmul(
                            ps, s6_sb[:, :], t5s[:, d : d + n, 1 : 1 + W],
                            start=True, stop=True,
                        )
                        yt = iopool.tile([OUT_C, n, W], f32, tag="yt")
                        nc.vector.tensor_scalar_add(yt, ps, b4_sb)
                        nc.scalar.dma_start(out=y_d[:, rr : rr + n, :], in_=yt)

                    out.append(emit)
                    rr += n
                return out

            pending = []  # tap-sum emitters from the previous strip
            for rep in range(repeat):
             for a in range(0, H, R):
                b = min(a + R, H)
                last = b == H
                lo_x, hi_x = max(0, a - 4), min(H, b + 4)

                if last:
                    # re-zero stale tail slots (bufs=1 reuse) before writes
                    nc.vector.memset(
                        x30[:, hi_x - a + 4 : R + 10, :].bitcast(f32), 0.0)
                    nc.vector.memset(
                        h1[:, 256 - (a - 3) : R + 6, :].bitcast(f32), 0.0)
                    nc.vector.memset(
                        h2[:, 256 - (a - 2) : R + 4, :].bitcast(f32), 0.0)
                    nc.vector.memset(
                        h3[:, 256 - (a - 1) : R + 2, :].bitcast(f32), 0.0)

                # --- load x strip, 5 shifted placements ---
                src = x_d[:, lo_x:hi_x, :].bitcast(f32r)
                o = lo_x - a
                nc.sync.dma_start(out=x30[0:6, o + 5 : hi_x - a + 5, 1 : 1 + W], in_=src)
                nc.sync.dma_start(out=x30[6:12, o + 6 : hi_x - a + 6, 1 : 1 + W], in_=src)
                nc.sync.dma_start(out=x30[12:18, o + 4 : hi_x - a + 4, 1 : 1 + W], in_=src)
                nc.sync.dma_start(out=x30[18:24, o + 5 : hi_x - a + 5, 2 : 2 + W], in_=src)
                nc.sync.dma_start(out=x30[24:30, o + 5 : hi_x - a + 5, 0:W], in_=src)

                # --- L1: rows [a-3, b+3) -> h1; copies alternate DVE/ACT ---
                rr = max(0, a - 3)
                hi = min(H, b + 3)
                ci = 0
                while rr < hi:
                    n = min(2, hi - rr)
                    s = rr - a + 5
                    ps = pmain.tile([HID_C, n, W], f32, tag="ps")
                    nc.tensor.matmul(
                        ps, w1_sb[:, :], x30[:, s : s + n, 1 : 1 + W],
                        start=True, stop=True,
                    )
                    d = rr - (a - 3)
                    if ci % 2 == 0:
                        nc.vector.tensor_scalar(
                            h1[:, d : d + n, 1 : 1 + W], ps, b1_sb, 0.0, Add, Max
                        )
                    else:
                        nc.scalar.activation(
                            h1[:, d : d + n, 1 : 1 + W], ps, Relu, bias=b1_sb
                        )
                    ci += 1
                    rr += n

                # --- L2: rows [a-2, b+2), reads h1; interleave prev tap-sum ---
                rr = max(0, a - 2) if stages >= 2 else hi
                hi = min(H, b + 2)
                while rr < hi:
                    n = min(2, hi - rr)
                    s = rr - a + 3  # h1 slot of center
                    ps = pmain.tile([HID_C, n, W], f32, tag="ps")
                    conv_chunk(ps, w2_sb, h1, s, n)
                    d = rr - (a - 2)
                    nc.scalar.activation(
                        h2[:, d : d + n, 1 : 1 + W], ps, Relu, bias=b2_sb
                    )
                    if pending and interleave:
                        pending.pop(0)()
                    rr += n
                while pending and interleave:
                    pending.pop(0)()

                # --- L3: rows [a-1, b+1), reads h2; mmA interleaved so
                # t5 fills (and the gather can start) as h3 rows land ---
                lo4, hi4 = max(0, a - 1), min(H, b + 1)
                mma_q = []
                rr = lo4
                ci4 = 0
                while rr < hi4:
                    n = min(2, hi4 - rr)
                    mma_q.append((rr, n))
                    rr += n

                def emit_mma(rr, n, ci):
                    s = rr - a + 1  # h3 slot of center
                    ps = pmain.tile([HID_C, n, W], f32, tag="ps")
                    nc.tensor.matmul(
                        ps, w4a_sb[:, :], h3[:, s : s + n, 1 : 1 + W],
                        start=True, stop=True,
                    )
                    d = rr - (a - 1)  # t5 slot
                    if ci % 2 == 0:
                        nc.vector.tensor_copy(t5[:, d : d + n, 1 : 1 + W], ps)
                    else:
                        nc.scalar.activation(
                            t5[:, d : d + n, 1 : 1 + W], ps, Ident)

                rr = max(0, a - 1) if stages >= 3 else H
                hi = min(H, b + 1)
                while rr < hi:
                    n = min(2, hi - rr)
                    s = rr - a + 2  # h2 slot of center
                    ps = pmain.tile([HID_C, n, W], f32, tag="ps")
                    conv_chunk(ps, w3_sb, h2, s, n)
                    d = rr - (a - 1)
                    nc.scalar.activation(
                        h3[:, d : d + n, 1 : 1 + W], ps, Relu, bias=b3_sb
                    )
                    if mma_q and mma_q[0][0] + 2 < rr:
                        r0, n0 = mma_q.pop(0)
                        emit_mma(r0, n0, ci4)
                        ci4 += 1
                    rr += n

                # --- L4a: drain remaining tap-partial chunks ---
                if stages < 4:
                    continue
                # tail slots beyond the written range must be zero (gathers
                # read them); t5 cells disjoint from mmA writes
                if hi4 - (a - 1) < R + 2:
                    nc.vector.memset(t5[:, hi4 - (a - 1) : R + 2, :], 0.0)
                while mma_q:
                    r0, n0 = mma_q.pop(0)
                    emit_mma(r0, n0, ci4)
                    ci4 += 1

                if stages < 5:
                    continue
                # --- gather: build pre-shifted t5s via SBUF->SBUF DMA ---
                # Full-padded-row flat copies: one contiguous run per
                # partition. t5 slot st = row-(a-1); t5s slot d = row-a.
                nr = b - a
                L = nr * WP
                t5f = t5.rearrange("p r c -> p (r c)")
                t5sf = t5s.rearrange("p r c -> p (r c)")
                nc.scalar.dma_start(out=t5sf[0:6, 0:L], in_=t5f[0:6, 0:L])
                nc.scalar.dma_start(out=t5sf[6:12, 0:L], in_=t5f[6:12, WP : WP + L])
                nc.scalar.dma_start(
                    out=t5sf[32:38, 0:L], in_=t5f[32:38, 2 * WP : 2 * WP + L])
                nc.scalar.dma_start(
                    out=t5sf[64:70, 1:L], in_=t5f[64:70, WP : WP + L - 1])
                nc.scalar.dma_start(
                    out=t5sf[96:102, 0 : L - 1], in_=t5f[96:102, WP + 1 : WP + L])

                if stages < 6:
                    continue
                if not interleave:
                    while pending:
                        pending.pop(0)()
                pending = l4b_chunks(a, b)

            # flush the final strip's tap-sum
            while pending:
                pending.pop(0)()

    nc.finalize()
    return nc


_NC_CACHE = {}


def _pack_inputs(x, w1, b1, w2, b2, w3, b3, w4, b4):
    x = np.ascontiguousarray(np.asarray(x, dtype=np.float32))
    w1 = np.asarray(w1, dtype=np.float32)
    w2 = np.asarray(w2, dtype=np.float32)
    w3 = np.asarray(w3, dtype=np.float32)
    w4 = np.asarray(w4, dtype=np.float32)
    # w4a slabs: up@0-5, center@6-11, down@32-37, left@64-69, right@96-101
    w4a = np.zeros((HID_C, HID_C), np.float32)
    slabs = ((0, 1), (6, 2 - 2), (32, 2), (64, 3), (96, 4))
    w4a[:, 0:OUT_C] = w4[:, :, 1].T          # up
    w4a[:, 6 : 6 + OUT_C] = w4[:, :, 0].T    # center
    w4a[:, 32 : 32 + OUT_C] = w4[:, :, 2].T  # down
    w4a[:, 64 : 64 + OUT_C] = w4[:, :, 3].T  # left
    w4a[:, 96 : 96 + OUT_C] = w4[:, :, 4].T  # right
    s6 = np.zeros((HID_C, OUT_C), np.float32)
    for base in (0, 6, 32, 64, 96):
        s6[base + np.arange(OUT_C), np.arange(OUT_C)] = 1.0
    s6 = s6.astype(ml_dtypes.bfloat16)
    common = {
        # w1p[t*6+ic, oc] = w1[oc, ic, t]
        "w1p": np.ascontiguousarray(w1.transpose(2, 1, 0).reshape(5 * IN_C, HID_C)),
        # w2p[ic, t, oc] = w2[oc, ic, t]
        "w2p": np.ascontiguousarray(w2.transpose(1, 2, 0)),
        "w3p": np.ascontiguousarray(w3.transpose(1, 2, 0)),
        "w4a": w4a,
        "s6": s6,
        "b1": np.asarray(b1, np.float32),
        "b2": np.asarray(b2, np.float32),
        "b3": np.asarray(b3, np.float32),
        "b4": np.asarray(b4, np.float32),
    }
    return x, common


def kernel(x, w1, b1, w2, b2, w3, b3, w4, b4):
    x, common = _pack_inputs(x, w1, b1, w2, b2, w3, b3, w4, b4)
    if "nc" not in _NC_CACHE:
        _NC_CACHE["nc"] = _build()
    nc = _NC_CACHE["nc"]
    in_maps = [dict(common, x=x[i]) for i in range(N_CORES)]
    res = bass_utils.run_bass_kernel_spmd(nc, in_maps, core_ids=list(range(N_CORES)))
    out = np.stack([res.results[i]["y"] for i in range(N_CORES)], axis=0)
    return out


# revision 22
# speedup vs baseline: 3.0203x; 1.0089x over previous
"""Trainium2 Bass kernel for 4-layer cross-stencil CNN.

Per-core: one image [6,256,256] (batch dim sharded across 8 cores).
conv(cross-5-stencil) = 5 channel-matmuls with spatially shifted rhs APs,
accumulated in PSUM. Channels on partitions, spatial (rows x cols) on the
free dim. fp32r matmuls (full PE rate at N>=256).

Strips of R output rows with overlap-compute for the halos; all four
layers fused in SBUF (no DRAM intermediates).

L1 packs the 5 taps into K=30 via a 5-group pre-shifted input buffer
(one matmul per chunk). L4 computes the 4 shifted taps as one M=128
matmul whose output slabs sit at partitions 0/32/64/96 (legal engine
bases), the center tap as an M=6 matmul, and the shifted tap-sum runs on
DVE in bf16; the tap-sum for strip k-1 is emitted interleaved into strip
k's L2 phase so the DVE FIFO never blocks the next strip's L1 copies.
"""

import sys

sys.path.insert(0, "/opt/trn_rl_repo")

import ml_dtypes
import numpy as np

import concourse.bacc as bacc
import concourse.mybir as mybir
from concourse.tile import TileContext
from concourse import bass_utils

IN_C, HID_C, OUT_C = 6, 128, 6
B, H, W = 8, 256, 256
WP = W + 2  # padded width
R = 24  # output rows per strip
N_CORES = 8

f32 = mybir.dt.float32
f32r = mybir.dt.float32r
bf16 = mybir.dt.bfloat16
Add = mybir.AluOpType.add
Max = mybir.AluOpType.max
Relu = mybir.ActivationFunctionType.Relu
Ident = mybir.ActivationFunctionType.Identity

# tap order matches reference: 0=center, 1=up(x[h-1]), 2=down(x[h+1]),
# 3=left(x[w-1]), 4=right(x[w+1])


def _build(repeat=1, stages=6, interleave=True):
    nc = bacc.Bacc("TRN2", target_bir_lowering=False)

    x_d = nc.dram_tensor("x", [IN_C, H, W], f32, kind="ExternalInput")
    w1_d = nc.dram_tensor("w1p", [5 * IN_C, HID_C], f32, kind="ExternalInput")
    w2_d = nc.dram_tensor("w2p", [HID_C, 5, HID_C], f32, kind="ExternalInput")
    w3_d = nc.dram_tensor("w3p", [HID_C, 5, HID_C], f32, kind="ExternalInput")
    # w4a: all 5 taps as M=128 slabs: up@0-5, center@6-11, down@32-37,
    # left@64-69, right@96-101; zero elsewhere
    w4a_d = nc.dram_tensor("w4a", [HID_C, HID_C], f32, kind="ExternalInput")
    # s6: bf16 selector summing the 5 (pre-shifted) slabs of t5s
    s6_d = nc.dram_tensor("s6", [HID_C, OUT_C], bf16, kind="ExternalInput")
    b1_d = nc.dram_tensor("b1", [HID_C], f32, kind="ExternalInput")
    b2_d = nc.dram_tensor("b2", [HID_C], f32, kind="ExternalInput")
    b3_d = nc.dram_tensor("b3", [HID_C], f32, kind="ExternalInput")
    b4_d = nc.dram_tensor("b4", [OUT_C], f32, kind="ExternalInput")
    y_d = nc.dram_tensor("y", [OUT_C, H, W], f32, kind="ExternalOutput")

    with TileContext(nc) as tc:
        with (
            tc.tile_pool(name="const", bufs=1) as cpool,
            tc.tile_pool(name="bufs", bufs=1) as bpool,
            tc.tile_pool(name="io", bufs=4) as iopool,
            tc.tile_pool(name="psmain", bufs=7, space="PSUM") as pmain,
        ):
            # --- weights / biases (resident) ---
            w1_sb = cpool.tile([5 * IN_C, HID_C], f32r)
            nc.sync.dma_start(out=w1_sb, in_=w1_d[:, :].bitcast(f32r))
            w2_sb = cpool.tile([HID_C, 5, HID_C], f32r)
            nc.sync.dma_start(out=w2_sb, in_=w2_d[:, :, :].bitcast(f32r))
            w3_sb = cpool.tile([HID_C, 5, HID_C], f32r)
            nc.sync.dma_start(out=w3_sb, in_=w3_d[:, :, :].bitcast(f32r))
            w4a_sb = cpool.tile([HID_C, HID_C], f32r)
            nc.sync.dma_start(out=w4a_sb, in_=w4a_d[:, :].bitcast(f32r))
            s6_sb = cpool.tile([HID_C, OUT_C], bf16)
            nc.sync.dma_start(out=s6_sb, in_=s6_d[:, :])
            b1_sb = cpool.tile([HID_C, 1], f32)
            nc.sync.dma_start(out=b1_sb, in_=b1_d[:, None])
            b2_sb = cpool.tile([HID_C, 1], f32)
            nc.sync.dma_start(out=b2_sb, in_=b2_d[:, None])
            b3_sb = cpool.tile([HID_C, 1], f32)
            nc.sync.dma_start(out=b3_sb, in_=b3_d[:, None])
            b4_sb = cpool.tile([OUT_C, 1], f32)
            nc.sync.dma_start(out=b4_sb, in_=b4_d[:, None])

            # --- persistent strip buffers (bufs=1; pads zeroed once) ---
            # x30: 5 tap-groups x 6ch, pre-shifted by DMA placement.
            # group g partitions [6g,6g+6); center x(h,w) -> (slot h-a+5, col w+1)
            x30 = bpool.tile([5 * IN_C, R + 10, WP], f32r)
            h1 = bpool.tile([HID_C, R + 6, WP], f32r)  # L1 rows [a-3,b+3)
            h2 = bpool.tile([HID_C, R + 4, WP], f32r)  # L2 rows [a-2,b+2)
            h3 = bpool.tile([HID_C, R + 2, WP], f32r)  # L3 rows [a-1,b+1)
            # t5: tap partials (slabs up@0,cen@6,dn@32,lf@64,rt@96), bf16
            t5 = bpool.tile([HID_C, R + 2, WP], bf16)
            # t5s: DMA-gathered pre-shifted taps; slot d = output row a+d
            t5s = bpool.tile([HID_C, R, WP], bf16)

            # zero only cells that are read but never written (all base-0 APs)
            nc.vector.memset(x30[:, 0:6, :].bitcast(f32), 0.0)
            nc.vector.memset(x30[:, :, 1:2].bitcast(f32), 0.0)
            nc.vector.memset(x30[:, :, 256:257].bitcast(f32), 0.0)
            for _t, _topz in ((h1, 3), (h2, 2), (h3, 1)):
                nc.vector.memset(_t[:, :, 0:1].bitcast(f32), 0.0)
                nc.vector.memset(_t[:, :, 257:258].bitcast(f32), 0.0)
                nc.vector.memset(_t[:, 0:_topz, :].bitcast(f32), 0.0)
            nc.vector.memset(t5[:, :, 0:1], 0.0)
            nc.vector.memset(t5[:, :, 257:258], 0.0)
            nc.vector.memset(t5[:, 0:1, :], 0.0)
            # t5s garbage partitions are never gathered; selector rows are
            # zero there but 0*garbage must not be NaN -> zero once
            nc.vector.memset(t5s[:, :, :], 0.0)

            def conv_chunk(ps, w_sb, src, s, n):
                """5 accumulating matmuls; center is src[:, s:s+n, 1:1+W]."""
                nc.tensor.matmul(
                    ps, w_sb[:, 0, :], src[:, s : s + n, 1 : 1 + W],
                    start=True, stop=False,
                )
                nc.tensor.matmul(
                    ps, w_sb[:, 1, :], src[:, s - 1 : s - 1 + n, 1 : 1 + W],
                    start=False, stop=False,
                )
                nc.tensor.matmul(
                    ps, w_sb[:, 2, :], src[:, s + 1 : s + 1 + n, 1 : 1 + W],
                    start=False, stop=False,
                )
                nc.tensor.matmul(
                    ps, w_sb[:, 3, :], src[:, s : s + n, 0:W],
                    start=False, stop=False,
                )
                nc.tensor.matmul(
                    ps, w_sb[:, 4, :], src[:, s : s + n, 2 : 2 + W],
                    start=False, stop=True,
                )

            def l4b_chunks(a0, b0):
                """Deferred emitters: one K=128 selector matmul sums the 5
                pre-shifted slabs of t5s; bias on DVE; DMA out."""
                out = []
                rr = a0
                while rr < b0:
                    n = min(2, b0 - rr)

                    def emit(rr=rr, n=n, a0=a0):
                        d = rr - a0
                        ps = pmain.tile([OUT_C, n, W], f32, tag="ps")
                        nc.tensor.matmul(
                            ps, s6_sb[:, :], t5s[:, d : d + n, 1 : 1 + W],
                            start=True, stop=True,
                        )
                        yt = iopool.tile([OUT_C, n, W], f32, tag="yt")
                        nc.vector.tensor_scalar_add(yt, ps, b4_sb)
                        nc.scalar.dma_start(out=y_d[:, rr : rr + n, :], in_=yt)

                    out.append(emit)
                    rr += n
                return out

            pending = []  # tap-sum emitters from the previous strip
            for rep in range(repeat):
             for a in range(0, H, R):
                b = min(a + R, H)
                last = b == H
                lo_x, hi_x = max(0, a - 4), min(H, b + 4)

                if last:
                    # re-zero stale tail slots (bufs=1 reuse) before writes
                    nc.vector.memset(
                        x30[:, hi_x - a + 4 : R + 10, :].bitcast(f32), 0.0)
                    nc.vector.memset(
                        h1[:, 256 - (a - 3) : R + 6, :].bitcast(f32), 0.0)
                    nc.vector.memset(
                        h2[:, 256 - (a - 2) : R + 4, :].bitcast(f32), 0.0)
                    nc.vector.memset(
                        h3[:, 256 - (a - 1) : R + 2, :].bitcast(f32), 0.0)

                # --- load x strip, 5 shifted placements ---
                src = x_d[:, lo_x:hi_x, :].bitcast(f32r)
                o = lo_x - a
                nc.sync.dma_start(out=x30[0:6, o + 5 : hi_x - a + 5, 1 : 1 + W], in_=src)
                nc.sync.dma_start(out=x30[6:12, o + 6 : hi_x - a + 6, 1 : 1 + W], in_=src)
                nc.sync.dma_start(out=x30[12:18, o + 4 : hi_x - a + 4, 1 : 1 + W], in_=src)
                nc.sync.dma_start(out=x30[18:24, o + 5 : hi_x - a + 5, 2 : 2 + W], in_=src)
                nc.sync.dma_start(out=x30[24:30, o + 5 : hi_x - a + 5, 0:W], in_=src)

                # --- L1: rows [a-3, b+3) -> h1; copies alternate DVE/ACT ---
                rr = max(0, a - 3)
                hi = min(H, b + 3)
                ci = 0
                while rr < hi:
                    n = min(2, hi - rr)
                    s = rr - a + 5
                    ps = pmain.tile([HID_C, n, W], f32, tag="ps")
                    nc.tensor.matmul(
                        ps, w1_sb[:, :], x30[:, s : s + n, 1 : 1 + W],
                        start=True, stop=True,
                    )
                    d = rr - (a - 3)
                    if ci % 2 == 0:
                        nc.vector.tensor_scalar(
                            h1[:, d : d + n, 1 : 1 + W], ps, b1_sb, 0.0, Add, Max
                        )
                    else:
                        nc.scalar.activation(
                            h1[:, d : d + n, 1 : 1 + W], ps, Relu, bias=b1_sb
                        )
                    ci += 1
                    rr += n

                # --- L2: rows [a-2, b+2), reads h1; interleave prev tap-sum ---
                rr = max(0, a - 2) if stages >= 2 else hi
                hi = min(H, b + 2)
                while rr < hi:
                    n = min(2, hi - rr)
                    s = rr - a + 3  # h1 slot of center
                    ps = pmain.tile([HID_C, n, W], f32, tag="ps")
                    conv_chunk(ps, w2_sb, h1, s, n)
                    d = rr - (a - 2)
                    nc.scalar.activation(
                        h2[:, d : d + n, 1 : 1 + W], ps, Relu, bias=b2_sb
                    )
                    if pending and interleave:
                        pending.pop(0)()
                    rr += n
                while pending and interleave:
                    pending.pop(0)()

                # --- L3: rows [a-1, b+1), reads h2; mmA interleaved so
                # t5 fills (and the gather can start) as h3 rows land ---
                lo4, hi4 = max(0, a - 1), min(H, b + 1)
                mma_q = []
                rr = lo4
                ci4 = 0
                while rr < hi4:
                    n = min(2, hi4 - rr)
                    mma_q.append((rr, n))
                    rr += n

                def emit_mma(rr, n, ci):
                    s = rr - a + 1  # h3 slot of center
                    ps = pmain.tile([HID_C, n, W], f32, tag="ps")
                    nc.tensor.matmul(
                        ps, w4a_sb[:, :], h3[:, s : s + n, 1 : 1 + W],
                        start=True, stop=True,
                    )
                    d = rr - (a - 1)  # t5 slot
                    if ci % 2 == 0:
                        nc.vector.tensor_copy(t5[:, d : d + n, 1 : 1 + W], ps)
                    else:
                        nc.scalar.activation(
                            t5[:, d : d + n, 1 : 1 + W], ps, Ident)

                rr = max(0, a - 1) if stages >= 3 else H
                hi = min(H, b + 1)
                while rr < hi:
                    n = min(2, hi - rr)
                    s = rr - a + 2  # h2 slot of center
                    ps = pmain.tile([HID_C, n, W], f32, tag="ps")
                    conv_chunk(ps, w3_sb, h2, s, n)
                    d = rr - (a - 1)
                    nc.scalar.activation(
                        h3[:, d : d + n, 1 : 1 + W], ps, Relu, bias=b3_sb
                    )
                    if mma_q and mma_q[0][0] + 2 < rr:
                        r0, n0 = mma_q.pop(0)
                        emit_mma(r0, n0, ci4)
                        ci4 += 1
                    rr += n

                # --- L4a: drain remaining tap-partial chunks ---
                if stages < 4:
                    continue
                # tail slots beyond the written range must be zero (gathers
                # read them); t5 cells disjoint from mmA writes
                if hi4 - (a - 1) < R + 2:
                    nc.vector.memset(t5[:, hi4 - (a - 1) : R + 2, :], 0.0)
                while mma_q:
                    r0, n0 = mma_q.pop(0)
                    emit_mma(r0, n0, ci4)
                    ci4 += 1

                if stages < 5:
                    continue
                # --- gather: build pre-shifted t5s via SBUF->SBUF DMA ---
                # Full-padded-row flat copies: one contiguous run per
                # partition. t5 slot st = row-(a-1); t5s slot d = row-a.
                nr = b - a
                t5f = t5.rearrange("p r c -> p (r c)")
                t5sf = t5s.rearrange("p r c -> p (r c)")
                # two half-strip gathers: the first half fires before the
                # last tap-partial copies land, unblocking the first
                # deferred sum-matmuls earlier in the next strip
                for h0, h1r in ((0, nr // 2), (nr // 2, nr)):
                    o0, o1 = h0 * WP, h1r * WP
                    nc.scalar.dma_start(
                        out=t5sf[0:6, o0:o1], in_=t5f[0:6, o0:o1])
                    nc.scalar.dma_start(
                        out=t5sf[6:12, o0:o1], in_=t5f[6:12, WP + o0 : WP + o1])
                    nc.scalar.dma_start(
                        out=t5sf[32:38, o0:o1],
                        in_=t5f[32:38, 2 * WP + o0 : 2 * WP + o1])
                    nc.scalar.dma_start(
                        out=t5sf[64:70, o0 + 1 : o1],
                        in_=t5f[64:70, WP + o0 : WP + o1 - 1])
                    nc.scalar.dma_start(
                        out=t5sf[96:102, o0 : o1 - 1],
                        in_=t5f[96:102, WP + o0 + 1 : WP + o1])

                if stages < 6:
                    continue
                if not interleave:
                    while pending:
                        pending.pop(0)()
                pending = l4b_chunks(a, b)

            # flush the final strip's tap-sum
            while pending:
                pending.pop(0)()

    nc.finalize()
    return nc


_NC_CACHE = {}


def _pack_inputs(x, w1, b1, w2, b2, w3, b3, w4, b4):
    x = np.ascontiguousarray(np.asarray(x, dtype=np.float32))
    w1 = np.asarray(w1, dtype=np.float32)
    w2 = np.asarray(w2, dtype=np.float32)
    w3 = np.asarray(w3, dtype=np.float32)
    w4 = np.asarray(w4, dtype=np.float32)
    # w4a slabs: up@0-5, center@6-11, down@32-37, left@64-69, right@96-101
    w4a = np.zeros((HID_C, HID_C), np.float32)
    slabs = ((0, 1), (6, 2 - 2), (32, 2), (64, 3), (96, 4))
    w4a[:, 0:OUT_C] = w4[:, :, 1].T          # up
    w4a[:, 6 : 6 + OUT_C] = w4[:, :, 0].T    # center
    w4a[:, 32 : 32 + OUT_C] = w4[:, :, 2].T  # down
    w4a[:, 64 : 64 + OUT_C] = w4[:, :, 3].T  # left
    w4a[:, 96 : 96 + OUT_C] = w4[:, :, 4].T  # right
    s6 = np.zeros((HID_C, OUT_C), np.float32)
    for base in (0, 6, 32, 64, 96):
        s6[base + np.arange(OUT_C), np.arange(OUT_C)] = 1.0
    s6 = s6.astype(ml_dtypes.bfloat16)
    common = {
        # w1p[t*6+ic, oc] = w1[oc, ic, t]
        "w1p": np.ascontiguousarray(w1.transpose(2, 1, 0).reshape(5 * IN_C, HID_C)),
        # w2p[ic, t, oc] = w2[oc, ic, t]
        "w2p": np.ascontiguousarray(w2.transpose(1, 2, 0)),
        "w3p": np.ascontiguousarray(w3.transpose(1, 2, 0)),
        "w4a": w4a,
        "s6": s6,
        "b1": np.asarray(b1, np.float32),
        "b2": np.asarray(b2, np.float32),
        "b3": np.asarray(b3, np.float32),
        "b4": np.asarray(b4, np.float32),
    }
    return x, common


def kernel(x, w1, b1, w2, b2, w3, b3, w4, b4):
    x, common = _pack_inputs(x, w1, b1, w2, b2, w3, b3, w4, b4)
    if "nc" not in _NC_CACHE:
        _NC_CACHE["nc"] = _build()
    nc = _NC_CACHE["nc"]
    in_maps = [dict(common, x=x[i]) for i in range(N_CORES)]
    res = bass_utils.run_bass_kernel_spmd(nc, in_maps, core_ids=list(range(N_CORES)))
    out = np.stack([res.results[i]["y"] for i in range(N_CORES)], axis=0)
    return out
